# revision 16
# baseline (speedup 1.0000x reference)
"""Trainium2 Bass kernel for nn_MoDBlock (mixture-of-depths block), 8 cores.

Contract: kernel(**inputs) takes FULL inputs (x (4,4096,2048) f32,
position_ids (4,4096) i32 [arange per spec], router_w, norm weights, qkv_w,
out_w, w1/w2/w3) and returns the FULL (4,4096,2048) f32 output.

Sharding: 4 pairs x 2 cores; pair g owns batch row b=g. Both cores of a pair
run the router (fp32 scores + tie), exact top-512 via gpsimd kth_largest ->
threshold -> sparse_gather compaction (ascending token order, matching
jax.lax.top_k + sort semantics incl. stable tie handling), and dma_gather of
the selected rows. Core half h processes selected ranks [256h, 256h+256):
q/attention-out/out-proj/SwiGLU for its ranks; k/v projections for all 512.
Causal mask on ranks == mask on original positions (positions ascending).

Precision: router + residuals fp32; attention core (q@k, softmax, @v) bf16;
all large matmuls (qkv, out-proj, w1/w2/w3) fp8e4m3 with DoubleRow perf mode
(2 k-tiles per instruction). Weights are quantized on host with per-matrix
scales (192/absmax); activations carry a fixed x8 scale folded into the
normalization constants; all dequants fold into existing PSUM-evacuation
copies via per-partition scale vectors (dq tile).
"""


import numpy as np
import ml_dtypes
import concourse.bass as bass
import concourse.bacc as bacc
import concourse.mybir as mybir
import concourse.tile as tile
from concourse import library_config
from concourse.tile_rust import add_dep_helper

F32 = mybir.dt.float32
BF16 = mybir.dt.bfloat16
F8 = mybir.dt.float8e4
AF = mybir.ActivationFunctionType
OP = mybir.AluOpType
DR = mybir.MatmulPerfMode.DoubleRow

B, T, D, H = 4, 4096, 2048, 16
HD = 128
K = 512
KC = 256          # tokens per core
DFF = 5461
DFFP = 5504       # padded to 43*128
NFC = DFFP // 128  # 43
NFC2 = 44          # padded to even for DoubleRow w3 contraction
EPS = 1e-6
ISQ = 1.0 / np.sqrt(128.0)
QUANT = 1.0 - 510.5 / 4095.0  # k_adj = 510 -> out {lerp, desc[511]}
SZ = 4.0   # fp8 scale for z = silu(u)*v
SH = 8.0   # fp8 scale for h1/h2/o activations


def build_kernel(tc: tile.TileContext, outs, ins):
    nc = tc.nc
    xb = ins["xb"]
    proc_o, idx_o, nf_o = outs["proc"], outs["idxo"], outs["nfo"]

    _open = {}

    def popen(name, side="left", **kw):
        cm = tc.tile_pool(name=name, side=side, **kw)
        _open[name] = cm
        return cm.__enter__()

    def pclose(name):
        _open.pop(name).__exit__(None, None, None)

    const = popen("const", bufs=1)
    small = popen("small", bufs=1)
    x1p = popen("x1_pool", bufs=1)          # lives A->H
    xown_pool = popen("xown_pool", bufs=1)  # lives A->E
    h1T_pool = popen("h1T_pool", side="right", bufs=1)  # lives A->C
    owpre = popen("owpre", bufs=1)          # ow8 preloaded during phase A
    rwp = popen("rw_pool", bufs=1)          # router weight, dies after A

    # full out-proj weight hoisted to SBUF via the Act engine's DMA queue:
    # it fills DMA-engine idle gaps (esp. the gpsimd selection window)
    # without ever blocking the sync queue's critical-path transfers
    owall = owpre.tile([128, 8, 8, 2, 256], F8, name="owall")
    nc.scalar.dma_start(owall[:], ins["ow8"][:])
    wvall = owpre.tile([128, 8, 8, 2, 256], F8, name="wvall")
    nc.scalar.dma_start(wvall[:], ins["wv8"][:])

    x1_t = x1p.tile([128, 2, 2048], F32, name="x1_t")
    xown_t = xown_pool.tile([128, 2, 2048], F32, name="xown_t")
    h1T8 = h1T_pool.tile([128, 16, 512], F8, name="h1T8")
    h1sel8 = h1T_pool.tile([128, 16, 256], F8, name="h1sel8")

    # ---- constants ----
    tie_t = const.tile([128, 32], F32)
    nc.sync.dma_start(tie_t[:], ins["tie"][:])
    iota_t = const.tile([128, 32], F32)
    nc.sync.dma_start(iota_t[:], ins["iota1"][:])
    ones1_t = const.tile([1, 128], F32)
    nc.sync.dma_start(ones1_t[:], ins["ones1"][:])
    ident_t = const.tile([128, 128], BF16)
    nc.sync.dma_start(ident_t[:], ins["identb"][:])
    # broadcast rw / norm weights across partitions on-chip (8KB DMA each
    # instead of 1MB): K=1 matmul ones[1,128].T @ row[1,512] per chunk
    row3_t = const.tile([1, 3, 2048], F32)
    nc.sync.dma_start(row3_t[:], ins["row3"][:])
    rw_t = rwp.tile([128, 2048], F32, name="rw_t")
    n1w_t = const.tile([128, 2048], BF16)
    n2w_t = const.tile([128, 2048], BF16)
    with tc.tile_pool(name="psB0", bufs=2, space="PSUM") as psB0:
        for i, dst in enumerate((rw_t, n1w_t, n2w_t)):
            for c in range(4):
                bc = psB0.tile([128, 512], F32, tag="bc", name=f"bc{i}_{c}")
                nc.tensor.matmul(
                    bc[:], ones1_t[:], row3_t[:, i, c * 512:(c + 1) * 512],
                    start=True, stop=True)
                nc.scalar.activation(dst[:, c * 512:(c + 1) * 512], bc[:],
                                     AF.Copy)
    cmask_t = const.tile([128, 4, 256], BF16)
    nc.sync.dma_start(cmask_t[:], ins["cmask"][:])
    qs0_t = const.tile([128, 1], F32)
    nc.sync.dma_start(qs0_t[:], ins["qs0"][:])
    qs1_t = const.tile([128, 1], F32)
    nc.sync.dma_start(qs1_t[:], ins["qs1"][:])
    dq_t = const.tile([128, 8], F32)
    nc.sync.dma_start(dq_t[:], ins["dq"][:])
    onesk_t = const.tile([128, 1], BF16)
    nc.vector.memset(onesk_t[:], 1.0)
    eps_t = const.tile([128, 1], F32)
    nc.vector.memset(eps_t[:], EPS)

    # =========== Phase A: router scores + topk + gather ===========
    S_t = small.tile([128, 32], F32)
    with tc.tile_pool(name="xstream", side="right", bufs=4) as xs:
        for k in range(32):
            xk = xs.tile([128, 2048], F32, tag="xk", name=f"xk{k}")
            nc.sync.dma_start(xk[:], xb[k * 128:(k + 1) * 128, :])
            nc.vector.scalar_tensor_tensor(
                out=xk[:], in0=xk[:], scalar=1.0, in1=rw_t[:],
                op0=OP.mult, op1=OP.mult, accum_out=S_t[:, k:k + 1],
            )
    nc.vector.tensor_add(out=S_t[:], in0=S_t[:], in1=tie_t[:])
    pclose("rw_pool")

    kth_t = small.tile([1, 2], F32)
    lib_attn = nc.gpsimd.load_library(library_config.attn)
    kth = nc.gpsimd.kth_largest(
        kth_t[:], S_t[:], n_per_lane=32, k=510, quantile=QUANT)
    add_dep_helper(kth.ins, lib_attn.ins, reason="lib attn first")

    th_t = small.tile([128, 1], F32)
    with tc.tile_pool(name="psA", bufs=1, space="PSUM") as psA:
        th_ps = psA.tile([128, 1], F32)
        nc.tensor.matmul(th_ps[:], ones1_t[:], kth_t[:, 1:2],
                         start=True, stop=True)
        nc.vector.tensor_copy(th_t[:], th_ps[:])

    cand_t = small.tile([128, 32], F32)
    nc.vector.scalar_tensor_tensor(
        out=cand_t[:], in0=S_t[:], scalar=th_t[:], in1=iota_t[:],
        op0=OP.is_ge, op1=OP.mult)
    nc.vector.tensor_scalar_add(cand_t[:], cand_t[:], -1.0)

    c16_t = small.tile([16, 32, 8], F32)
    for pi in range(8):
        nc.sync.dma_start(c16_t[:, :, pi], cand_t[pi * 16:(pi + 1) * 16, :])

    sg_t = small.tile([16, 33], F32)
    nf_t = small.tile([1, 1], mybir.dt.uint32)
    lib_sg = nc.gpsimd.load_library(library_config.sparse_gather)
    sg = nc.gpsimd.sparse_gather(
        sg_t[:], c16_t[:].rearrange("p k j -> p (k j)"), num_found=nf_t[:])
    add_dep_helper(lib_sg.ins, kth.ins, reason="lib switch after kth")
    add_dep_helper(sg.ins, lib_sg.ins, reason="sg after lib")
    nc.sync.dma_start(nf_o[:], nf_t[:])

    idx32_t = small.tile([16, 32], mybir.dt.int32)
    nc.vector.tensor_copy(idx32_t[:], sg_t[:, 0:32])
    nc.sync.dma_start(idx_o.rearrange("(f p) -> p f", p=16), idx32_t[:])

    idx16_t = small.tile([16, 32], mybir.dt.int16)
    nc.vector.tensor_copy(idx16_t[:], sg_t[:, 0:32])
    idx128_t = small.tile([128, 32], mybir.dt.int16)
    for g in range(8):
        nc.sync.dma_start(idx128_t[g * 16:(g + 1) * 16, :], idx16_t[:])

    x_sel = popen("x_sel_pool", bufs=1)
    xsel_t = x_sel.tile([128, 4, 2048], F32, name="xsel_t")
    lib_mlp = nc.gpsimd.load_library(library_config.mlp)
    gat = nc.gpsimd.dma_gather(
        xsel_t[:], xb[:], idx128_t[:], K, K, 2048)
    add_dep_helper(lib_mlp.ins, sg.ins, reason="lib switch after sg")
    add_dep_helper(gat.ins, lib_mlp.ins, reason="gather after lib")

    # =========== Phase B: norm1, h1 (x8), h1T8, blends ===========
    rs1_t = small.tile([128, 4], F32)
    sq1_t = small.tile([128, 4], F32)
    with tc.tile_pool(name="scratch", bufs=1) as scr:
        for c in range(4):
            sc = scr.tile([128, 2048], F32, tag="sc", name=f"sc{c}")
            nc.vector.scalar_tensor_tensor(
                out=sc[:], in0=xsel_t[:, c, :], scalar=1.0,
                in1=xsel_t[:, c, :], op0=OP.mult, op1=OP.mult,
                accum_out=sq1_t[:, c:c + 1])
    nc.scalar.activation(rs1_t[:], sq1_t[:], AF.Sqrt,
                         scale=1.0 / 2048.0, bias=eps_t[:])
    nc.vector.reciprocal(rs1_t[:], rs1_t[:])
    # fold the x8 fp8 activation scale into rs1
    nc.vector.tensor_scalar_mul(rs1_t[:], rs1_t[:], SH)

    with (
        tc.tile_pool(name="h1_pool", bufs=1) as h1p,
        tc.tile_pool(name="psT", bufs=4, space="PSUM") as psT,
    ):
        h1_t = h1p.tile([128, 4, 2048], BF16, name="h1_t")
        for c in range(4):
            nc.vector.scalar_tensor_tensor(
                out=h1_t[:, c, :], in0=xsel_t[:, c, :],
                scalar=rs1_t[:, c:c + 1], in1=n1w_t[:],
                op0=OP.mult, op1=OP.mult)
        for c in range(4):
            for dc in range(16):
                pt = psT.tile([128, 128], BF16, tag="pt", name=f"pt{c}_{dc}")
                nc.tensor.transpose(
                    pt[:], h1_t[:, c, dc * 128:(dc + 1) * 128], ident_t[:])
                nc.scalar.activation(
                    h1T8[:, dc, c * 128:(c + 1) * 128], pt[:], AF.Copy)

    # x_own / h1sel blends (qs0/qs1 are {0,1} select masks)
    for qt in range(2):
        nc.vector.tensor_scalar_mul(
            xown_t[:, qt, :], xsel_t[:, qt, :], qs0_t[:])
        nc.vector.scalar_tensor_tensor(
            out=xown_t[:, qt, :], in0=xsel_t[:, 2 + qt, :],
            scalar=qs1_t[:], in1=xown_t[:, qt, :],
            op0=OP.mult, op1=OP.add)
    hselb = h1T_pool.tile([128, 16, 256], BF16, name="hselb")
    for dc in range(16):
        nc.vector.tensor_scalar_mul(
            hselb[:, dc, :], h1T8[:, dc, 0:256], qs0_t[:])
        nc.vector.scalar_tensor_tensor(
            out=h1sel8[:, dc, :], in0=h1T8[:, dc, 256:512],
            scalar=qs1_t[:], in1=hselb[:, dc, :],
            op0=OP.mult, op1=OP.add)
    pclose("x_sel_pool")

    # =========== Phase C: qkv projections (fp8 DoubleRow) ===========
    qkvp = popen("qkv_pool", bufs=1)
    qT = qkvp.tile([128, 16, 256], BF16, name="qT")
    kT = qkvp.tile([128, 16, 512], BF16, name="kT")
    # V laid out per head with a trailing ones column (129 = 128 vdims + 1)
    # so the attention av matmul also produces the softmax denominator
    V = qkvp.tile([128, 4, 16, 129], BF16, name="V")
    nc.vector.memset(V[:, :, :, 128:129], 1.0)

    with (
        tc.tile_pool(name="wqk_stream", bufs=5) as wqs,
        tc.tile_pool(name="psC", bufs=2, space="PSUM") as psC,
    ):
        for jc in range(16):
            wqc = wqs.tile([128, 16, 128], F8, tag="wqc", name=f"wq{jc}")
            nc.sync.dma_start(wqc[:], ins["wq8"][:, jc])
            pq = psC.tile([128, 256], F32, tag="pq", bufs=2, name=f"pq{jc}")
            for dp in range(8):
                nc.tensor.matmul(pq[:], wqc[:, 2 * dp:2 * dp + 2, :],
                                 h1sel8[:, 2 * dp:2 * dp + 2, :],
                                 start=(dp == 0), stop=(dp == 7),
                                 perf_mode=DR)
            nc.scalar.activation(qT[:, jc, :], pq[:], AF.Copy,
                                 scale=dq_t[:, 0:1])
        for jc in range(16):
            wkc = wqs.tile([128, 16, 128], F8, tag="wqc", name=f"wk{jc}")
            nc.sync.dma_start(wkc[:], ins["wk8"][:, jc])
            for kh in range(2):
                pk = psC.tile([128, 256], F32, tag="pk", bufs=2,
                              name=f"pk{jc}_{kh}")
                for dp in range(8):
                    nc.tensor.matmul(
                        pk[:], wkc[:, 2 * dp:2 * dp + 2, :],
                        h1T8[:, 2 * dp:2 * dp + 2,
                             kh * 256:(kh + 1) * 256],
                        start=(dp == 0), stop=(dp == 7), perf_mode=DR)
                nc.scalar.activation(kT[:, jc, kh * 256:(kh + 1) * 256],
                                     pk[:], AF.Copy, scale=dq_t[:, 1:2])
        for vc in range(8):
            wvc = wvall[:, vc]
            pvs = [psC.tile([128, 256], F32, tag="pv", bufs=4,
                            name=f"pv{vc}_{i}") for i in range(4)]
            for dp in range(8):
                for tc4 in range(4):
                    nc.tensor.matmul(
                        pvs[tc4][:],
                        h1T8[:, 2 * dp:2 * dp + 2,
                             tc4 * 128:(tc4 + 1) * 128],
                        wvc[:, dp], start=(dp == 0), stop=(dp == 7),
                        perf_mode=DR)
            for tc4 in range(4):
                nc.scalar.activation(
                    V[:, tc4, 2 * vc:2 * vc + 2, 0:128], pvs[tc4][:],
                    AF.Copy, scale=dq_t[:, 2:3])
    pclose("h1T_pool")

    # =========== Phase D: attention (bf16) ===========
    attp = popen("att_pool", side="right", bufs=1)
    o_t = attp.tile([128, 2, 16, 128], BF16, name="o_t")
    oT8 = attp.tile([128, 16, 256], F8, name="oT8")
    with (
        tc.tile_pool(name="pT_pool", bufs=2) as pTp,
        tc.tile_pool(name="lrow_pool", bufs=2) as lrp,
        tc.tile_pool(name="psD", bufs=2, space="PSUM") as psD,
    ):
        for h in range(16):
            pT = pTp.tile([128, 4, 256], BF16, tag="pT", name=f"pT{h}")
            for kc in range(4):
                ss = psD.tile([128, 256], F32, tag="ss", name=f"ss{h}_{kc}")
                nc.tensor.matmul(
                    ss[:], kT[:, h, kc * 128:(kc + 1) * 128], qT[:, h, :],
                    start=True, stop=True)
                pe_t = pTp.tile([128, 256], F32, tag="pe", name=f"pe{h}_{kc}")
                nc.scalar.activation(pe_t[:], ss[:], AF.Exp, scale=ISQ)
                nc.vector.tensor_mul(
                    out=pT[:, kc, :], in0=pe_t[:], in1=cmask_t[:, kc, :])
            for qt in range(2):
                # av matmul over [vdims | ones] -> col 128 is the softmax
                # denominator (pT already carries the causal mask)
                po = psD.tile([128, 129], F32, tag="po", name=f"po{h}_{qt}")
                for kc in range(4):
                    nc.tensor.matmul(
                        po[:], pT[:, kc, qt * 128:(qt + 1) * 128],
                        V[:, kc, h, :],
                        start=(kc == 0), stop=(kc == 3))
                rL = lrp.tile([128, 1], F32, tag="rL", name=f"rL{h}_{qt}")
                nc.vector.reciprocal(rL[:], po[:, 128:129])
                nc.scalar.activation(o_t[:, qt, h, :], po[:, 0:128],
                                     AF.Copy, scale=rL[:, 0:1])
    with tc.tile_pool(name="psT2", bufs=2, space="PSUM") as psT2:
        for qt in range(2):
            for h in range(16):
                pt = psT2.tile([128, 128], BF16, tag="pt2",
                               name=f"pt2_{qt}_{h}")
                nc.tensor.transpose(pt[:], o_t[:, qt, h, :], ident_t[:])
                nc.scalar.activation(
                    oT8[:, h, qt * 128:(qt + 1) * 128], pt[:], AF.Copy,
                    scale=SH)
    pclose("qkv_pool")

    # =========== Phase E: out-proj (fp8 DR) + residual -> x1 ===========
    with (
        tc.tile_pool(name="psE", bufs=2, space="PSUM") as psE,
    ):
        for nk in range(8):
            owc = owall[:, nk]
            for qt in range(2):
                poo = psE.tile([128, 256], F32, tag="poo",
                               name=f"poo{nk}_{qt}")
                for op_ in range(8):
                    nc.tensor.matmul(
                        poo[:],
                        oT8[:, 2 * op_:2 * op_ + 2,
                            qt * 128:(qt + 1) * 128],
                        owc[:, op_], start=(op_ == 0), stop=(op_ == 7),
                        perf_mode=DR)
                nc.vector.scalar_tensor_tensor(
                    out=x1_t[:, qt, nk * 256:(nk + 1) * 256],
                    in0=poo[:], scalar=dq_t[:, 3:4],
                    in1=xown_t[:, qt, nk * 256:(nk + 1) * 256],
                    op0=OP.mult, op1=OP.add)
    pclose("att_pool")
    pclose("owpre")
    pclose("xown_pool")

    # =========== Phase F: norm2 + h2T8 ===========
    rs2_t = small.tile([128, 2], F32)
    sq2_t = small.tile([128, 2], F32)
    zzp = popen("zz_pool", side="right", bufs=1)
    zz = zzp.tile([128, NFC2, 256], F8, name="zz")
    h2Tp = popen("h2T_pool", side="right", bufs=1)
    h2T8 = h2Tp.tile([128, 16, 256], F8, name="h2T8")

    with tc.tile_pool(name="scratch2", bufs=2) as scr2:
        for c in range(2):
            sc = scr2.tile([128, 2048], F32, tag="sc2", name=f"sc2_{c}")
            nc.vector.scalar_tensor_tensor(
                out=sc[:], in0=x1_t[:, c, :], scalar=1.0,
                in1=x1_t[:, c, :], op0=OP.mult, op1=OP.mult,
                accum_out=sq2_t[:, c:c + 1])
    nc.scalar.activation(rs2_t[:], sq2_t[:], AF.Sqrt,
                         scale=1.0 / 2048.0, bias=eps_t[:])
    nc.vector.reciprocal(rs2_t[:], rs2_t[:])
    nc.vector.tensor_scalar_mul(rs2_t[:], rs2_t[:], SH)

    with (
        tc.tile_pool(name="h2_pool", bufs=1) as h2p,
        tc.tile_pool(name="psT3", bufs=2, space="PSUM") as psT3,
    ):
        h2_t = h2p.tile([128, 2, 2048], BF16, name="h2_t")
        for c in range(2):
            nc.vector.scalar_tensor_tensor(
                out=h2_t[:, c, :], in0=x1_t[:, c, :],
                scalar=rs2_t[:, c:c + 1], in1=n2w_t[:],
                op0=OP.mult, op1=OP.mult)
        for c in range(2):
            for dc in range(16):
                pt = psT3.tile([128, 128], BF16, tag="pt3",
                               name=f"pt3_{c}_{dc}")
                nc.tensor.transpose(
                    pt[:], h2_t[:, c, dc * 128:(dc + 1) * 128], ident_t[:])
                nc.scalar.activation(
                    h2T8[:, dc, c * 128:(c + 1) * 128], pt[:], AF.Copy)

    # =========== Phase G: FFN w1/w2 (fp8 DR) -> zz ===========
    nc.vector.memset(zz[:, NFC2 - 1, :], 0.0)
    with (
        tc.tile_pool(name="w12_stream", bufs=5) as w12s,
        tc.tile_pool(name="sig_pool", bufs=3) as sigp,
        tc.tile_pool(name="psG", bufs=2, space="PSUM") as psG,
    ):
        for fc in range(NFC):
            w1c = w12s.tile([128, 16, 128], F8, tag="w1c", name=f"w1c{fc}")
            nc.sync.dma_start(w1c[:], ins["w18"][:, fc])
            w2c = w12s.tile([128, 16, 128], F8, tag="w2c", name=f"w2c{fc}")
            nc.sync.dma_start(w2c[:], ins["w28"][:, fc])
            p1 = psG.tile([128, 256], F32, tag="p1", name=f"p1_{fc}")
            p2 = psG.tile([128, 256], F32, tag="p2", name=f"p2_{fc}")
            for dp in range(8):
                nc.tensor.matmul(p1[:], w1c[:, 2 * dp:2 * dp + 2, :],
                                 h2T8[:, 2 * dp:2 * dp + 2, :],
                                 start=(dp == 0), stop=(dp == 7),
                                 perf_mode=DR)
            for dp in range(8):
                nc.tensor.matmul(p2[:], w2c[:, 2 * dp:2 * dp + 2, :],
                                 h2T8[:, 2 * dp:2 * dp + 2, :],
                                 start=(dp == 0), stop=(dp == 7),
                                 perf_mode=DR)
            sg2 = sigp.tile([128, 256], BF16, tag="sg2", name=f"sg2_{fc}")
            nc.scalar.activation(sg2[:], p1[:], AF.Sigmoid,
                                 scale=dq_t[:, 4:5])
            s1 = sigp.tile([128, 256], BF16, tag="s1", name=f"s1_{fc}")
            nc.vector.scalar_tensor_tensor(
                out=s1[:], in0=sg2[:], scalar=dq_t[:, 4:5], in1=p1[:],
                op0=OP.mult, op1=OP.mult)
            nc.vector.scalar_tensor_tensor(
                out=zz[:, fc, :], in0=s1[:], scalar=dq_t[:, 5:6], in1=p2[:],
                op0=OP.mult, op1=OP.mult)
    pclose("h2T_pool")

    # =========== Phase H: w3 (fp8 DR) + residual -> proc ===========
    procp = popen("proc_pool", bufs=1)
    proc_t = procp.tile([128, 2, 2048], BF16, name="proc_t")
    with (
        tc.tile_pool(name="w3_stream", bufs=5) as w3s,
        tc.tile_pool(name="psH", bufs=1, space="PSUM") as psH,
    ):
        pffs = {}
        for qt in range(2):
            for np_ in range(4):
                pffs[(qt, np_)] = psH.tile(
                    [128, 512], F32, tag=f"pff{qt}{np_}",
                    name=f"pff{qt}{np_}")
        NJ = NFC2 // 2
        for j in range(NJ):
            w3c = w3s.tile([128, 2, 2048], F8, tag="w3c", name=f"w3c{j}")
            nc.sync.dma_start(w3c[:], ins["w38"][:, j])
            for qt in range(2):
                for np_ in range(4):
                    nc.tensor.matmul(
                        pffs[(qt, np_)][:],
                        zz[:, 2 * j:2 * j + 2, qt * 128:(qt + 1) * 128],
                        w3c[:, :, np_ * 512:(np_ + 1) * 512],
                        start=(j == 0), stop=(j == NJ - 1),
                        perf_mode=DR)
        for qt in range(2):
            for np_ in range(4):
                nc.vector.scalar_tensor_tensor(
                    out=proc_t[:, qt, np_ * 512:(np_ + 1) * 512],
                    in0=pffs[(qt, np_)][:], scalar=dq_t[:, 6:7],
                    in1=x1_t[:, qt, np_ * 512:(np_ + 1) * 512],
                    op0=OP.mult, op1=OP.add)
    for qt in range(2):
        nc.sync.dma_start(proc_o[qt * 128:(qt + 1) * 128, :],
                          proc_t[:, qt, :])
    pclose("proc_pool")
    pclose("zz_pool")
    pclose("x1_pool")
    pclose("small")
    pclose("const")


# ======================= host side =======================

def host_constants(inputs):
    """Shared per-core constants from full inputs (numpy)."""
    f32 = np.float32
    bf = ml_dtypes.bfloat16
    e4 = ml_dtypes.float8_e4m3
    qkv_w = np.asarray(inputs["qkv_w"], f32)
    con = {}
    row3 = np.stack([np.asarray(inputs["router_w"], f32),
                     np.asarray(inputs["norm1_w"], f32),
                     np.asarray(inputs["norm2_w"], f32)])[None]
    con["row3"] = np.ascontiguousarray(row3)  # [1, 3, 2048]
    tie = (np.arange(T, dtype=f32) * np.float32(1e-6))
    con["tie"] = tie.reshape(32, 128).T.copy()
    con["iota1"] = (np.arange(T, dtype=f32) + 1.0).reshape(32, 128).T.copy().astype(f32)
    con["ones1"] = np.ones((1, 128), f32)
    con["identb"] = np.eye(128, dtype=f32).astype(bf)

    def q8(w, s):
        return np.ascontiguousarray((np.asarray(w, f32) * s)).astype(e4)

    def wscale(w):
        return f32(192.0 / np.abs(np.asarray(w, f32)).max())

    wq = qkv_w[:, 0:2048]
    wk = qkv_w[:, 2048:4096]
    wv = qkv_w[:, 4096:6144]
    ow = np.asarray(inputs["out_w"], f32)
    w1 = np.zeros((2048, DFFP), f32)
    w1[:, :DFF] = np.asarray(inputs["w1"], f32)
    w2 = np.zeros((2048, DFFP), f32)
    w2[:, :DFF] = np.asarray(inputs["w2"], f32)
    w3 = np.zeros((NFC2 * 128, 2048), f32)
    w3[:DFF, :] = np.asarray(inputs["w3"], f32)
    s_wq, s_wk, s_wv = wscale(wq), wscale(wk), wscale(wv)
    s_ow, s_w1, s_w2, s_w3 = wscale(ow), wscale(w1), wscale(w2), wscale(w3)

    # device layouts (partition-contiguous):
    # wq8/wk8: [p, jc, dc, 128] = w[dc*128+p, jc*128+j]
    wq4 = q8(wq, s_wq).reshape(16, 128, 16, 128)        # dc p jc j
    con["wq8"] = np.ascontiguousarray(wq4.transpose(1, 2, 0, 3))
    wk4 = q8(wk, s_wk).reshape(16, 128, 16, 128)
    con["wk8"] = np.ascontiguousarray(wk4.transpose(1, 2, 0, 3))
    # wv8: [p, vc, dp, i, n] = wv[(2dp+i)*128+p, vc*256+n]
    wv4 = q8(wv, s_wv).reshape(8, 2, 128, 8, 256)       # dp i p vc n
    con["wv8"] = np.ascontiguousarray(wv4.transpose(2, 3, 0, 1, 4))
    # ow8: [p, nk, op, i, n] = ow[(2op+i)*128+p, nk*256+n]
    ow4 = q8(ow, s_ow).reshape(8, 2, 128, 8, 256)
    con["ow8"] = np.ascontiguousarray(ow4.transpose(2, 3, 0, 1, 4))
    # w18/w28: [p, fc, dc, f] = w[dc*128+p, fc*128+f]
    w14 = q8(w1, s_w1).reshape(16, 128, NFC, 128)
    con["w18"] = np.ascontiguousarray(w14.transpose(1, 2, 0, 3))
    w24 = q8(w2, s_w2).reshape(16, 128, NFC, 128)
    con["w28"] = np.ascontiguousarray(w24.transpose(1, 2, 0, 3))
    # w38: [p, j, i, n] = w3[(2j+i)*128+p, n]
    w34 = q8(w3, s_w3).reshape(NFC2 // 2, 2, 128, 2048)
    con["w38"] = np.ascontiguousarray(w34.transpose(2, 0, 1, 3))

    dq = np.zeros((128, 8), f32)
    dq[:, 0] = 1.0 / (SH * s_wq)
    dq[:, 1] = 1.0 / (SH * s_wk)
    dq[:, 2] = 1.0 / (SH * s_wv)
    dq[:, 3] = 1.0 / (SH * s_ow)
    dq[:, 4] = 1.0 / (SH * s_w1)
    dq[:, 5] = SZ / (SH * s_w2)
    dq[:, 6] = 1.0 / (SZ * s_w3)
    con["dq"] = dq
    return con


def host_core_inputs(inputs, con, c):
    f32 = np.float32
    bf = ml_dtypes.bfloat16
    b, half = c // 2, c % 2
    qoff = half * KC
    m = dict(con)
    m["xb"] = np.ascontiguousarray(np.asarray(inputs["x"], f32)[b])
    # causal multiplicative mask on ranks: [4kc][128k, 256q]: 1 if k_rank <= qoff+q
    kr = np.arange(K)[:, None]
    qr = (qoff + np.arange(KC))[None, :]
    mask = (kr <= qr).astype(f32).reshape(4, 128, KC).transpose(1, 0, 2)
    m["cmask"] = np.ascontiguousarray(mask).astype(bf)
    m["qs0"] = np.full((128, 1), 1.0 - half, f32)
    m["qs1"] = np.full((128, 1), float(half), f32)
    return m


_BUILT = None


def _build_program():
    global _BUILT
    if _BUILT is not None:
        return _BUILT
    nc = bacc.Bacc("TRN2", target_bir_lowering=False, debug=False,
                   enable_asserts=True, num_devices=8)
    in_specs = {
        "xb": ((T, D), F32), "row3": ((1, 3, 2048), F32),
        "tie": ((128, 32), F32), "iota1": ((128, 32), F32),
        "ones1": ((1, 128), F32), "identb": ((128, 128), BF16),
        "cmask": ((128, 4, 256), BF16),
        "qs0": ((128, 1), F32), "qs1": ((128, 1), F32),
        "dq": ((128, 8), F32),
        "wq8": ((128, 16, 16, 128), F8), "wk8": ((128, 16, 16, 128), F8),
        "wv8": ((128, 8, 8, 2, 256), F8), "ow8": ((128, 8, 8, 2, 256), F8),
        "w18": ((128, NFC, 16, 128), F8), "w28": ((128, NFC, 16, 128), F8),
        "w38": ((128, NFC2 // 2, 2, 2048), F8),
    }
    out_specs = {
        "proc": ((KC, D), BF16), "idxo": ((K,), mybir.dt.int32),
        "nfo": ((1, 1), mybir.dt.uint32),
    }
    ins = {k: nc.dram_tensor(k, s, d, kind="ExternalInput").ap()
           for k, (s, d) in in_specs.items()}
    outs = {k: nc.dram_tensor(k, s, d, kind="ExternalOutput").ap()
            for k, (s, d) in out_specs.items()}
    with tile.TileContext(nc) as tc:
        build_kernel(tc, outs, ins)
    nc.compile()
    _BUILT = nc
    return nc


def kernel(**inputs):
    from concourse import bass_utils
    from concourse.bass_interp import get_hw_module

    nc = _build_program()
    con = host_constants(inputs)
    in_maps = [host_core_inputs(inputs, con, c) for c in range(8)]

    old_m = nc.m
    nc.m = get_hw_module(nc.m)
    try:
        res = bass_utils.run_bass_kernel_spmd(
            nc, in_maps, core_ids=list(range(8)))
    finally:
        nc.m = old_m

    x = np.asarray(inputs["x"], np.float32)
    out = x.copy()
    for g in range(B):
        idx = np.asarray(res.results[2 * g]["idxo"]).astype(np.int64)
        proc0 = np.asarray(res.results[2 * g]["proc"])
        proc1 = np.asarray(res.results[2 * g + 1]["proc"])
        out[g, idx[0:KC]] = proc0.astype(np.float32)
        out[g, idx[KC:K]] = proc1.astype(np.float32)
    return out
